# revision 1
# baseline (speedup 1.0000x reference)
"""HGNN conv kernel for Trainium2, data-parallel over time across 8 cores.

Per core (t = core index): out_b = Dv^-1/2 Gc De^-1 Gc^T Dv^-1/2 (x_b W + 1 b^T)
computed in factored form (L never materialized):
  Gs  = Dv^-1/2 Gc                      [N, E]
  zT  = x_t^T Gs  per 128-row bf block  [BF, E]   (MM1)
  zw  = zT^T-blocks @ blockdiag(W,W) + u0 bias^T  [E, BF]  (W-MM + fused bias)
  out = Gsd^T zw with Gsd = de * Gs^T   [N, BF]   (MM2)
All matmuls run in float32r (full PE rate, ~1e-4 rel err).
"""

import sys

import numpy as np

sys.path.insert(0, "/opt/trn_rl_repo")

from contextlib import ExitStack

import concourse.bass as bass
import concourse.mybir as mybir
import concourse.tile as tile
from concourse import bacc, bass_utils
from concourse.masks import make_identity

P = 128
T = 8
B = 28          # batch entries per core
N = 1024        # nodes
E = 512         # hyperedges (256 static + 256 dynamic)
F = 64          # features
BF = B * F      # 1792
EPS = 1e-6
NT = N // P     # 8 n-tiles
ET = E // P     # 4 e-tiles
MT = BF // P    # 14 bf-tiles (2 batch entries each)
NB = 4          # output free-dim chunks
NBW = BF // NB  # 448 = 7 batch entries * 64

f32 = mybir.dt.float32
f32r = mybir.dt.float32r


def _build_nc():
    nc = bacc.Bacc("TRN2", target_bir_lowering=False, debug=False)

    xs = nc.dram_tensor("xs", [B, N, F], f32, kind="ExternalInput").ap()
    g = nc.dram_tensor("g", [N, 256], f32, kind="ExternalInput").ap()
    g1 = nc.dram_tensor("g1", [N, 256], f32, kind="ExternalInput").ap()
    w = nc.dram_tensor("w", [F, F], f32, kind="ExternalInput").ap()
    bvec = nc.dram_tensor("b", [F], f32, kind="ExternalInput").ap()
    os_ = nc.dram_tensor("os", [B, N, F], f32, kind="ExternalOutput").ap()

    with tile.TileContext(nc) as tc, ExitStack() as ctx:
        const = ctx.enter_context(tc.tile_pool(name="const", bufs=1))
        big = ctx.enter_context(tc.tile_pool(name="big", bufs=1))
        ztp = ctx.enter_context(tc.tile_pool(name="ztp", bufs=3))
        osb = ctx.enter_context(tc.tile_pool(name="osb", bufs=4))
        ps_stats = ctx.enter_context(tc.tile_pool(name="ps_stats", bufs=1, space="PSUM"))
        ps_small = ctx.enter_context(tc.tile_pool(name="ps_small", bufs=2, space="PSUM"))
        ps_z = ctx.enter_context(tc.tile_pool(name="ps_z", bufs=2, space="PSUM"))
        ps_o = ctx.enter_context(tc.tile_pool(name="ps_o", bufs=2, space="PSUM"))

        # ---- input loads -------------------------------------------------
        # x slice as [n-part, k(n-tile), b, f], cast to f32r during DMA
        xs_all = big.tile([P, NT, B, F], f32r, name="xs_all")
        xs_r = xs.rearrange("b (k p) f -> p k b f", p=P)
        for k in range(NT):
            nc.gpsimd.dma_start(xs_all[:, k], xs_r[:, k])

        # Gc = [G | G1] as [n-part, k, e], cast to f32r during DMA
        gc_all = big.tile([P, NT, E], f32r, name="gc_all")
        nc.gpsimd.dma_start(gc_all[:, :, 0:256], g.rearrange("(k p) e -> p k e", p=P))
        nc.gpsimd.dma_start(gc_all[:, :, 256:512], g1.rearrange("(k p) e -> p k e", p=P))

        # blockdiag(W, W) [128, 128] f32r
        bdw_f = const.tile([P, P], f32, name="bdw_f")
        nc.vector.memset(bdw_f[:], 0.0)
        nc.sync.dma_start(bdw_f[0:64, 0:64], w)
        nc.sync.dma_start(bdw_f[64:128, 64:128], w)
        bdw = const.tile([P, P], f32r, name="bdw")
        nc.vector.tensor_copy(bdw[:], bdw_f[:])

        # bias tiled twice [1, 128] f32r
        btmp = const.tile([1, F], f32, name="btmp")
        nc.sync.dma_start(btmp[:], bvec[None, :])
        bias2 = const.tile([1, 2, F], f32r, name="bias2")
        nc.vector.tensor_copy(bias2[:], btmp[0:1, None, :].to_broadcast([1, 2, F]))
        bias_bc = const.tile([P, P], f32r, name="bias_bc")
        nc.gpsimd.partition_broadcast(
            bias_bc[:], bias2[:].rearrange("o t f -> o (t f)")
        )

        ident_f = const.tile([P, P], f32, name="ident_f")
        make_identity(nc, ident_f[:])
        ident = const.tile([P, P], f32r, name="ident")
        nc.vector.tensor_copy(ident[:], ident_f[:])

        # ---- degree stats ------------------------------------------------
        # dv = 1/sqrt(rowsum(Gc) + eps)   [128, NT]
        rs = const.tile([P, NT], f32, name="rs")
        for k in range(NT):
            nc.vector.reduce_sum(rs[:, k : k + 1], gc_all[:, k, :], axis=mybir.AxisListType.X)
        eps_col = const.tile([P, 1], f32, name="eps_col")
        nc.vector.memset(eps_col[:], EPS)
        sq = const.tile([P, NT], f32, name="sq")
        nc.scalar.activation(
            sq[:], rs[:], mybir.ActivationFunctionType.Sqrt, bias=eps_col[:]
        )
        dv = const.tile([P, NT], f32, name="dv")
        nc.vector.reciprocal(dv[:], sq[:])

        # lhsT per k-tile: [ones | dv_k] -> colsums of Gc (row 0) and Gs (row 1)
        onesdv_f = const.tile([P, NT, 2], f32, name="onesdv_f")
        nc.vector.memset(onesdv_f[:, :, 0:1], 1.0)
        nc.vector.tensor_copy(onesdv_f[:, :, 1:2], dv[:, :, None])
        onesdv = const.tile([P, NT, 2], f32r, name="onesdv")
        nc.vector.tensor_copy(onesdv[:], onesdv_f[:])
        stats_ps = ps_stats.tile([2, E], f32, name="stats_ps")
        for k in range(NT):
            nc.tensor.matmul(
                stats_ps[:], onesdv[:, k, :], gc_all[:, k, :],
                start=(k == 0), stop=(k == NT - 1),
            )
        stats_sb = const.tile([2, E], f32r, name="stats_sb")
        nc.vector.tensor_copy(stats_sb[:], stats_ps[:])

        # transpose stats to column layout [128, ET, 2] = [cs | u0]
        statsT = const.tile([P, ET, 2], f32, name="statsT")
        for j in range(ET):
            tp = ps_small.tile([P, P], f32r, name="sp")[:, 0:2]
            nc.tensor.matmul(
                tp[:], stats_sb[:, j * P : (j + 1) * P], ident[0:2, 0:2],
                is_transpose=True,
            )
            nc.vector.tensor_copy(statsT[:, j, :], tp[:])
        de_col = const.tile([P, ET], f32, name="de_col")
        nc.vector.tensor_scalar(
            out=de_col[:], in0=statsT[:, :, 0], scalar1=EPS, scalar2=None,
            op0=mybir.AluOpType.add,
        )
        nc.vector.reciprocal(de_col[:], de_col[:])

        # ---- Gs and Gsd --------------------------------------------------
        gs_all = big.tile([P, NT, E], f32r, name="gs_all")
        for k in range(NT):
            nc.vector.tensor_scalar(
                out=gs_all[:, k, :], in0=gc_all[:, k, :], scalar1=dv[:, k : k + 1],
                scalar2=None, op0=mybir.AluOpType.mult,
            )

        # Gsd[e, n] = de[e] * Gs[n, e] via PE transpose + scaled evict
        gsd_all = big.tile([P, ET, N], f32r, name="gsd_all")
        for j in range(ET):
            for i in range(NT):
                tp = ps_small.tile([P, P], f32r, name="sp")
                nc.tensor.matmul(
                    tp[:], gs_all[:, i, j * P : (j + 1) * P], ident[:],
                    is_transpose=True,
                )
                nc.vector.tensor_scalar(
                    out=gsd_all[:, j, i * P : (i + 1) * P], in0=tp[:],
                    scalar1=de_col[:, j : j + 1], scalar2=None,
                    op0=mybir.AluOpType.mult,
                )

        # ---- MM1 + W-MM pipeline ----------------------------------------
        # v_all[e-part, j, bf'] = de-unscaled zw + u0*bias  (f32r)
        v_all = big.tile([P, ET, BF], f32r, name="v_all")
        xs_flat = xs_all[:].rearrange("p k b f -> p k (b f)")

        for m in range(MT):
            zps = ps_z.tile([P, E], f32, name="zps")
            for k in range(NT):
                nc.tensor.matmul(
                    zps[:], xs_flat[:, k, m * P : (m + 1) * P], gs_all[:, k, :],
                    start=(k == 0), stop=(k == NT - 1),
                )
            zt = ztp.tile([P, E], f32r, name="zt")
            nc.scalar.copy(zt[:], zps[:])
            for j in range(ET):
                wps = ps_small.tile([P, P], f32, name="sp")
                nc.tensor.matmul(
                    wps[:], zt[:, j * P : (j + 1) * P], bdw[:],
                    start=True, stop=True,
                )
                # v = (bias_bcast * u0_col) + zw_psum, rounded to f32r
                nc.vector.scalar_tensor_tensor(
                    out=v_all[:, j, m * P : (m + 1) * P],
                    in0=bias_bc[:],
                    scalar=statsT[:, j, 1:2],
                    in1=wps[:],
                    op0=mybir.AluOpType.mult,
                    op1=mybir.AluOpType.add,
                )

        # ---- MM2 + store -------------------------------------------------
        os_r = os_.rearrange("b (i p) f -> p i b f", p=P)
        for i in range(NT):
            for nb in range(NB):
                ops = ps_o.tile([P, NBW], f32, name="ops")
                for j in range(ET):
                    nc.tensor.matmul(
                        ops[:], gsd_all[:, j, i * P : (i + 1) * P],
                        v_all[:, j, nb * NBW : (nb + 1) * NBW],
                        start=(j == 0), stop=(j == ET - 1),
                    )
                ot = osb.tile([P, NBW], f32, name="ot")
                if (i * NB + nb) % 2 == 0:
                    nc.scalar.copy(ot[:], ops[:])
                else:
                    nc.vector.tensor_copy(ot[:], ops[:])
                nc.sync.dma_start(
                    os_r[:, i, nb * 7 : (nb + 1) * 7, :],
                    ot[:].rearrange("p (c f) -> p c f", f=F),
                )

    nc.finalize()
    return nc


_NC = None


def _get_nc():
    global _NC
    if _NC is None:
        _NC = _build_nc()
    return _NC


def kernel(x, G, G1, weight, bias):
    nc = _get_nc()
    x = np.ascontiguousarray(x, dtype=np.float32)
    G = np.ascontiguousarray(G, dtype=np.float32)
    G1 = np.ascontiguousarray(G1, dtype=np.float32)
    weight = np.ascontiguousarray(weight, dtype=np.float32)
    bias = np.ascontiguousarray(bias, dtype=np.float32)

    in_maps = []
    for c in range(T):
        in_maps.append(
            {
                "xs": x[c * B : (c + 1) * B],
                "g": G,
                "g1": np.ascontiguousarray(G1[c]),
                "w": weight,
                "b": bias,
            }
        )
    res = bass_utils.run_bass_kernel_spmd(nc, in_maps, core_ids=list(range(T)))
    return np.concatenate([r["os"] for r in res.results], axis=0)



# revision 12
# speedup vs baseline: 1.2968x; 1.2968x over previous
"""HGNN conv kernel for Trainium2, data-parallel over time across 8 cores.

Per core (t = core index): out_b = Dv^-1/2 Gc De^-1 Gc^T Dv^-1/2 (x_b W + 1 b^T)
computed in factored form (L never materialized):
  Gs  = Dv^-1/2 Gc                       [N, E]   (bf16)
  z   = x^T Gs per 128-col bf block      [BF, E]  (MM1)
  zw  = z^T-blocks @ blockdiag(W,W)      [E, BF]  (W-MM; + u0 (x) bias add)
  out = Gsd^T v with Gsd = de * Gs^T     [N, BF]  (MM2)

All matmul operands are bf16 (fp32-family moving operands stream at ~2.4
cycles/col on TRN2 PE vs 1 for bf16).  Nodes are swizzled n = p*8 + n2 so one
SBUF partition covers 8 consecutive DRAM rows -> 2KB-contiguous DMA
descriptors for x loads (via an f32 staging buffer + cast-permute on GpSimd)
and for out stores (via an SBUF staging buffer).  x arrives in 14 batch-pair
chunks so MM1 starts as soon as the degree stats are ready; MM2 runs on
512-wide bf column chunks interleaved into the m loop so output DMA overlaps
compute.  A few dummy matmuls at t=0 warm the PE HAM clock to 2.4 GHz.
"""

import sys

import numpy as np

sys.path.insert(0, "/opt/trn_rl_repo")

from contextlib import ExitStack

import concourse.bass as bass
import concourse.mybir as mybir
import concourse.tile as tile
from concourse import bacc, bass_utils
from concourse.masks import make_identity

P = 128
T = 8
B = 28          # batch entries per core
N = 1024        # nodes
E = 512         # hyperedges (256 static + 256 dynamic)
F = 64          # features
BF = B * F      # 1792
EPS = 1e-6
N2 = 8          # node swizzle: n = p*8 + n2
MT = BF // P    # 14 bf-tiles (2 batch entries each)
ET = E // P     # 4 e-tiles
# MM2 output column chunks (bf columns); last one is the short tail
CHUNKS = [(0, 512), (512, 1024), (1024, 1536), (1536, 1792)]

f32 = mybir.dt.float32
f32r = mybir.dt.float32r
bf16 = mybir.dt.bfloat16


def _build_nc():
    nc = bacc.Bacc("TRN2", target_bir_lowering=False, debug=False)

    xs = nc.dram_tensor("xs", [B, N, F], f32, kind="ExternalInput").ap()
    g = nc.dram_tensor("g", [N, 256], f32r, kind="ExternalInput").ap()
    g1 = nc.dram_tensor("g1", [N, 256], f32r, kind="ExternalInput").ap()
    w = nc.dram_tensor("w", [F, F], f32, kind="ExternalInput").ap()
    bvec = nc.dram_tensor("b", [F], f32, kind="ExternalInput").ap()
    os_ = nc.dram_tensor("os", [B, N, F], f32, kind="ExternalOutput").ap()

    with tile.TileContext(nc) as tc, ExitStack() as ctx:
        const = ctx.enter_context(tc.tile_pool(name="const", bufs=1))
        big = ctx.enter_context(tc.tile_pool(name="big", bufs=1))
        xstage = ctx.enter_context(tc.tile_pool(name="xstage", bufs=3))
        ztp = ctx.enter_context(tc.tile_pool(name="ztp", bufs=3))
        osb = ctx.enter_context(tc.tile_pool(name="osb", bufs=2))
        # PSUM: 8 banks, bank-granular per site x bufs:
        # zps 2 + stats 1 + wps(+warm) 1 + ops 2 + transposes 2 = 8
        ps_z = ctx.enter_context(tc.tile_pool(name="ps_z", bufs=2, space="PSUM"))
        ps_st = ctx.enter_context(tc.tile_pool(name="ps_st", bufs=1, space="PSUM"))
        ps_w = ctx.enter_context(tc.tile_pool(name="ps_w", bufs=1, space="PSUM"))
        ps_o = ctx.enter_context(tc.tile_pool(name="ps_o", bufs=2, space="PSUM"))
        ps_t = ctx.enter_context(tc.tile_pool(name="ps_t", bufs=2, space="PSUM"))

        # ---- input DMA (HWDGE on sync queue) -----------------------------
        bdw_f = const.tile([P, P], f32, name="bdw_f")
        nc.vector.memset(bdw_f[:], 0.0)
        nc.sync.dma_start(bdw_f[0:64, 0:64], w)
        nc.sync.dma_start(bdw_f[64:128, 64:128], w)
        btmp = const.tile([1, F], f32, name="btmp")
        nc.sync.dma_start(btmp[:], bvec[None, :])

        # Gc = [G | G1] as [p, n2, e] with n = p*8 + n2; 8 chunked DMAs so
        # the rowsum pipeline starts early
        gc_all = big.tile([P, N2, E], f32r, name="gc_all")
        g_r = g.rearrange("(p n2) e -> p n2 e", p=P)
        g1_r = g1.rearrange("(p n2) e -> p n2 e", p=P)
        for h in range(4):
            nc.sync.dma_start(gc_all[:, 2 * h : 2 * h + 2, 0:256], g_r[:, 2 * h : 2 * h + 2])
            nc.sync.dma_start(gc_all[:, 2 * h : 2 * h + 2, 256:512], g1_r[:, 2 * h : 2 * h + 2])

        # x: DMA lands [p, b, n2, f] f32 staging (2KB descriptors); GpSimd
        # cast+permutes to [p, n2, b, f] bf16 so MM1 lhsT slices are a single
        # contiguous 128-elem free dim (matmul cannot mix 32-bit and bf16)
        xs_all = big.tile([P, N2, B, F], bf16, name="xs_all")
        xs_r = xs.rearrange("b (p n2) f -> p b n2 f", p=P)
        xstages = []
        for m in range(MT):
            xf = xstage.tile([P, 2, N2, F], f32, name="xf")
            nc.sync.dma_start(xf[:], xs_r[:, 2 * m : 2 * m + 2])
            xstages.append(xf)

        def cast_chunk(m):
            nc.gpsimd.tensor_copy(
                xs_all[:, :, 2 * m : 2 * m + 2, :],
                xstages[m][:].rearrange("p b k f -> p k b f"),
            )

        # ---- constants / PE warmup --------------------------------------
        ident_f = const.tile([P, P], f32, name="ident_f")
        make_identity(nc, ident_f[:])
        ident_b = const.tile([P, P], bf16, name="ident_b")
        nc.vector.tensor_copy(ident_b[:], ident_f[:])

        warm_sb = const.tile([P, 512], bf16, name="warm_sb")
        nc.vector.memset(warm_sb[:], 0.0)
        warm_ps = ps_w.tile([P, 512], f32, name="wps")
        for i in range(8):
            nc.tensor.matmul(
                warm_ps[:], ident_b[:], warm_sb[:], start=(i == 0), stop=(i == 7)
            )

        bdw = const.tile([P, P], bf16, name="bdw")
        nc.vector.tensor_copy(bdw[:], bdw_f[:])

        bias2 = const.tile([1, 2, F], f32, name="bias2")
        nc.vector.tensor_copy(bias2[:], btmp[0:1, None, :].to_broadcast([1, 2, F]))
        bias_bc = const.tile([P, P], f32, name="bias_bc")
        nc.gpsimd.partition_broadcast(
            bias_bc[:], bias2[:].rearrange("o t f -> o (t f)")
        )

        # ---- degree stats ------------------------------------------------
        # dv = 1/sqrt(rowsum(Gc) + eps)   [128, N2]; split DVE/ACT
        rs = const.tile([P, N2], f32, name="rs")
        rs_junk = const.tile([P, E], f32, name="rs_junk")
        for k in range(N2):
            if k % 2 == 0:
                nc.vector.reduce_sum(
                    rs[:, k : k + 1], gc_all[:, k, :], axis=mybir.AxisListType.X
                )
            else:
                nc.scalar.activation(
                    rs_junk[:], gc_all[:, k, :],
                    mybir.ActivationFunctionType.Copy,
                    accum_out=rs[:, k : k + 1],
                )
        eps_col = const.tile([P, 1], f32, name="eps_col")
        nc.vector.memset(eps_col[:], EPS)
        sq = const.tile([P, N2], f32, name="sq")
        nc.scalar.activation(
            sq[:], rs[:], mybir.ActivationFunctionType.Sqrt, bias=eps_col[:]
        )
        dv = const.tile([P, N2], f32, name="dv")
        nc.vector.reciprocal(dv[:], sq[:])

        # lhsT per n2: [ones | dv] -> colsums of Gc (row 0) and Gs (row 1)
        onesdv_f = const.tile([P, N2, 2], f32, name="onesdv_f")
        nc.vector.memset(onesdv_f[:, :, 0:1], 1.0)
        nc.vector.tensor_copy(onesdv_f[:, :, 1:2], dv[:, :, None])
        onesdv = const.tile([P, N2, 2], f32r, name="onesdv")
        nc.vector.tensor_copy(onesdv[:], onesdv_f[:])
        stats_ps = ps_st.tile([2, E], f32, name="stats_ps")
        for k in range(N2):
            nc.tensor.matmul(
                stats_ps[:], onesdv[:, k, :], gc_all[:, k, :],
                start=(k == 0), stop=(k == N2 - 1),
            )
        stats_sb = const.tile([2, E], bf16, name="stats_sb")
        nc.vector.tensor_copy(stats_sb[:], stats_ps[:])

        # transpose stats to column layout [128, ET, 2] = [cs | u0]
        statsT = const.tile([P, ET, 2], f32, name="statsT")
        for j in range(ET):
            tp = ps_t.tile([P, P], bf16, name="sp")[:, 0:2]
            nc.tensor.matmul(
                tp[:], stats_sb[:, j * P : (j + 1) * P], ident_b[0:2, 0:2],
                is_transpose=True,
            )
            nc.vector.tensor_copy(statsT[:, j, :], tp[:])
        de_col = const.tile([P, ET], f32, name="de_col")
        nc.vector.tensor_scalar(
            out=de_col[:], in0=statsT[:, :, 0], scalar1=EPS, scalar2=None,
            op0=mybir.AluOpType.add,
        )
        nc.vector.reciprocal(de_col[:], de_col[:])

        # ub[e-part, j, bf2] = u0[e] * bias-pattern  (added to every zw m-tile)
        ub = const.tile([P, ET, P], f32, name="ub")
        for j in range(ET):
            nc.vector.tensor_scalar(
                out=ub[:, j, :], in0=bias_bc[:], scalar1=statsT[:, j, 1:2],
                scalar2=None, op0=mybir.AluOpType.mult,
            )

        # ---- Gs (bf16) and Gsd -------------------------------------------
        gs_all = big.tile([P, N2, E], bf16, name="gs_all")
        for k in range(N2):
            nc.vector.tensor_scalar(
                out=gs_all[:, k, :], in0=gc_all[:, k, :], scalar1=dv[:, k : k + 1],
                scalar2=None, op0=mybir.AluOpType.mult,
            )

        # Gsd[e, n-col] = de[e] * Gs[n, e] via PE transpose + scaled evict
        # n-col order is (n2, q): col n2*128+q holds n = q*8 + n2
        gsd_all = big.tile([P, ET, N], bf16, name="gsd_all")

        def gsd_strip(k):
            for j in range(ET):
                tp = ps_t.tile([P, P], bf16, name="sp")
                nc.tensor.matmul(
                    tp[:], gs_all[:, k, j * P : (j + 1) * P], ident_b[:],
                    is_transpose=True,
                )
                nc.vector.tensor_scalar(
                    out=gsd_all[:, j, k * P : (k + 1) * P], in0=tp[:],
                    scalar1=de_col[:, j : j + 1], scalar2=None,
                    op0=mybir.AluOpType.mult,
                )

        # ---- main pipeline ----------------------------------------------
        # v_all[e-part, j, bf] = zw + u0*bias  (bf16)
        v_all = big.tile([P, ET, BF], bf16, name="v_all")
        os_r = os_.rearrange("b (p n2) f -> p b n2 f", p=P)

        def mm1(m):
            zps = ps_z.tile([P, E], f32, name="zps")
            for k in range(N2):
                nc.tensor.matmul(
                    zps[:], xs_all[:, k, 2 * m : 2 * m + 2, :], gs_all[:, k, :],
                    start=(k == 0), stop=(k == N2 - 1),
                )
            return zps

        def wmm(m, zps):
            zt = ztp.tile([P, E], bf16, name="zt")
            nc.scalar.copy(zt[:], zps[:])
            wps = ps_w.tile([P, E], f32, name="wps")
            for j in range(ET):
                nc.tensor.matmul(
                    wps[:, j * P : (j + 1) * P], zt[:, j * P : (j + 1) * P], bdw[:],
                    start=True, stop=True,
                )
            # v = ub + zw for all 4 j-blocks in one DVE op
            nc.vector.scalar_tensor_tensor(
                out=v_all[:, :, m * P : (m + 1) * P],
                in0=ub[:],
                scalar=1.0,
                in1=wps[:].rearrange("p (j c) -> p j c", j=ET),
                op0=mybir.AluOpType.mult,
                op1=mybir.AluOpType.add,
            )

        def mm2(c):
            c0, c1 = CHUNKS[c]
            nb = (c1 - c0) // F  # batch entries in this chunk
            ob = osb.tile([P, 8, N2, F], f32, name="ob")
            for k in range(N2):
                ops = ps_o.tile([P, 512], f32, name="ops")[:, 0 : c1 - c0]
                for j in range(ET):
                    nc.tensor.matmul(
                        ops[:], gsd_all[:, j, k * P : (k + 1) * P],
                        v_all[:, j, c0:c1],
                        start=(j == 0), stop=(j == ET - 1),
                    )
                dst = ob[:, 0:nb, k, :]
                src = ops[:].rearrange("p (c f) -> p c f", f=F)
                if k % 2 == 0:
                    nc.scalar.copy(dst, src)
                else:
                    nc.vector.tensor_copy(dst, src)
            nc.scalar.dma_start(
                os_r[:, c0 // F : c1 // F], ob[:, 0:nb, :, :]
            )

        cast_chunk(0)
        cast_chunk(1)
        for k in range(N2):
            gsd_strip(k)
        zps_prev = mm1(0)
        for m in range(1, MT):
            cast_chunk(m + 1) if m + 1 < MT else None
            zps = mm1(m)
            wmm(m - 1, zps_prev)
            zps_prev = zps
            if m in (5, 9, 13):
                mm2(m // 4 - 1)
        wmm(MT - 1, zps_prev)
        mm2(3)

    nc.finalize()
    return nc


_NC = None


def _get_nc():
    global _NC
    if _NC is None:
        _NC = _build_nc()
    return _NC


def kernel(x, G, G1, weight, bias):
    nc = _get_nc()
    x = np.ascontiguousarray(x, dtype=np.float32)
    G = np.ascontiguousarray(G, dtype=np.float32)
    G1 = np.ascontiguousarray(G1, dtype=np.float32)
    weight = np.ascontiguousarray(weight, dtype=np.float32)
    bias = np.ascontiguousarray(bias, dtype=np.float32)

    in_maps = []
    for c in range(T):
        in_maps.append(
            {
                "xs": x[c * B : (c + 1) * B],
                "g": G,
                "g1": np.ascontiguousarray(G1[c]),
                "w": weight,
                "b": bias,
            }
        )
    res = bass_utils.run_bass_kernel_spmd(nc, in_maps, core_ids=list(range(T)))
    return np.concatenate([r["os"] for r in res.results], axis=0)


# revision 18
# speedup vs baseline: 1.3834x; 1.0668x over previous
"""HGNN conv kernel for Trainium2, data-parallel over time across 8 cores.

Per core (t = core index): out_b = Dv^-1/2 Gc De^-1 Gc^T Dv^-1/2 (x_b W + 1 b^T)
computed in factored form (L never materialized):
  Gs  = Dv^-1/2 Gc                       [N, E]   (bf16)
  z   = x^T Gs per 128-col bf block      [BF, E]  (MM1)
  zw  = z^T-blocks @ blockdiag(W,W)      [E, BF]  (W-MM; + u0 (x) bias add)
  out = Gsd^T v with Gsd = de * Gs^T     [N, BF]  (MM2)

All matmul operands are bf16 (fp32-family moving operands stream at ~2.4
cycles/col on TRN2 PE vs 1 for bf16).  Nodes are swizzled n = p*8 + n2 so one
SBUF partition covers 8 consecutive DRAM rows -> 2KB-contiguous DMA
descriptors for x loads (via an f32 staging buffer + cast-permute on GpSimd)
and for out stores (via an SBUF staging buffer).  x arrives in 14 batch-pair
chunks so MM1 starts as soon as the degree stats are ready; MM2 runs on
512-wide bf column chunks interleaved into the m loop so output DMA overlaps
compute.  A few dummy matmuls at t=0 warm the PE HAM clock to 2.4 GHz.
"""

import sys

import numpy as np

sys.path.insert(0, "/opt/trn_rl_repo")

from contextlib import ExitStack

import concourse.bass as bass
import concourse.mybir as mybir
import concourse.tile as tile
from concourse import bacc, bass_utils
from concourse.masks import make_identity

P = 128
T = 8
B = 28          # batch entries per core
N = 1024        # nodes
E = 512         # hyperedges (256 static + 256 dynamic)
F = 64          # features
BF = B * F      # 1792
EPS = 1e-6
N2 = 8          # node swizzle: n = p*8 + n2
MT = BF // P    # 14 bf-tiles (2 batch entries each)
ET = E // P     # 4 e-tiles
# MM2 output column chunks (bf columns); tail chunks shrink so the last
# MM2+evict+store after the final m-tile is short
CHUNKS = [(0, 512), (512, 1024), (1024, 1408), (1408, 1664), (1664, 1792)]
# emit chunk c right after wmm(m) for m = CHUNK_AFTER[c]
CHUNK_AFTER = {4: 0, 8: 1, 11: 2, 12: 3, 13: 4}

f32 = mybir.dt.float32
f32r = mybir.dt.float32r
bf16 = mybir.dt.bfloat16


def _build_nc():
    nc = bacc.Bacc("TRN2", target_bir_lowering=False, debug=False)

    xs = nc.dram_tensor("xs", [B, N, F], f32, kind="ExternalInput").ap()
    g = nc.dram_tensor("g", [N, 256], f32r, kind="ExternalInput").ap()
    g1 = nc.dram_tensor("g1", [N, 256], f32r, kind="ExternalInput").ap()
    w = nc.dram_tensor("w", [F, F], f32, kind="ExternalInput").ap()
    bvec = nc.dram_tensor("b", [F], f32, kind="ExternalInput").ap()
    os_ = nc.dram_tensor("os", [B, N, F], f32, kind="ExternalOutput").ap()

    with tile.TileContext(nc) as tc, ExitStack() as ctx:
        const = ctx.enter_context(tc.tile_pool(name="const", bufs=1))
        big = ctx.enter_context(tc.tile_pool(name="big", bufs=1))
        ztp = ctx.enter_context(tc.tile_pool(name="ztp", bufs=3))
        osb = ctx.enter_context(tc.tile_pool(name="osb", bufs=2))
        # PSUM: 8 banks, bank-granular per site x bufs:
        # zps 2 + stats 1 + wps(+warm) 1 + ops 2 + transposes 2 = 8
        ps_z = ctx.enter_context(tc.tile_pool(name="ps_z", bufs=2, space="PSUM"))
        ps_st = ctx.enter_context(tc.tile_pool(name="ps_st", bufs=1, space="PSUM"))
        ps_w = ctx.enter_context(tc.tile_pool(name="ps_w", bufs=1, space="PSUM"))
        ps_o = ctx.enter_context(tc.tile_pool(name="ps_o", bufs=2, space="PSUM"))
        ps_t = ctx.enter_context(tc.tile_pool(name="ps_t", bufs=2, space="PSUM"))

        # ---- input DMA (HWDGE on sync queue) -----------------------------
        bdw_f = const.tile([P, P], f32, name="bdw_f")
        nc.vector.memset(bdw_f[:], 0.0)
        nc.sync.dma_start(bdw_f[0:64, 0:64], w)
        nc.sync.dma_start(bdw_f[64:128, 64:128], w)
        btmp = const.tile([1, F], f32, name="btmp")
        nc.sync.dma_start(btmp[:], bvec[None, :])

        # Gc = [G | G1] as [p, n2, e] with n = p*8 + n2; 8 chunked DMAs so
        # the rowsum pipeline starts early
        gc_all = big.tile([P, N2, E], f32r, name="gc_all")
        g_r = g.rearrange("(p n2) e -> p n2 e", p=P)
        g1_r = g1.rearrange("(p n2) e -> p n2 e", p=P)
        for h in range(4):
            nc.sync.dma_start(gc_all[:, 2 * h : 2 * h + 2, 0:256], g_r[:, 2 * h : 2 * h + 2])
            nc.sync.dma_start(gc_all[:, 2 * h : 2 * h + 2, 256:512], g1_r[:, 2 * h : 2 * h + 2])

        # x lands directly in MM1 layout [p, n2, b, f] bf16 via SWDGE cast
        # DMAs (matmul cannot mix 32-bit and bf16), one DMA per (b-group of
        # 8, n2) so MM1 m-tiles unblock as batch groups arrive
        xs_all = big.tile([P, N2, B, F], bf16, name="xs_all")
        xs_r2 = xs.rearrange("b (p n2) f -> p n2 b f", p=P)
        for gb in range(4):
            bsl = slice(8 * gb, min(8 * gb + 8, B))
            for k in range(N2):
                nc.gpsimd.dma_start(xs_all[:, k, bsl, :], xs_r2[:, k, bsl, :])

        # ---- constants / PE warmup --------------------------------------
        ident_f = const.tile([P, P], f32, name="ident_f")
        make_identity(nc, ident_f[:])
        ident_b = const.tile([P, P], bf16, name="ident_b")
        nc.vector.tensor_copy(ident_b[:], ident_f[:])

        warm_sb = const.tile([P, 512], bf16, name="warm_sb")
        nc.vector.memset(warm_sb[:], 0.0)
        warm_ps = ps_w.tile([P, 512], f32, name="wps")
        for i in range(8):
            nc.tensor.matmul(
                warm_ps[:], ident_b[:], warm_sb[:], start=(i == 0), stop=(i == 7)
            )

        bdw = const.tile([P, P], bf16, name="bdw")
        nc.vector.tensor_copy(bdw[:], bdw_f[:])

        bias2 = const.tile([1, 2, F], f32, name="bias2")
        nc.vector.tensor_copy(bias2[:], btmp[0:1, None, :].to_broadcast([1, 2, F]))
        bias_bc = const.tile([P, P], f32, name="bias_bc")
        nc.gpsimd.partition_broadcast(
            bias_bc[:], bias2[:].rearrange("o t f -> o (t f)")
        )

        # ---- degree stats ------------------------------------------------
        # dv = 1/sqrt(rowsum(Gc) + eps)   [128, N2]; split DVE/ACT
        rs = const.tile([P, N2], f32, name="rs")
        rs_junk = const.tile([P, E], f32, name="rs_junk")
        for k in range(N2):
            if k % 2 == 0:
                nc.vector.reduce_sum(
                    rs[:, k : k + 1], gc_all[:, k, :], axis=mybir.AxisListType.X
                )
            else:
                nc.scalar.activation(
                    rs_junk[:], gc_all[:, k, :],
                    mybir.ActivationFunctionType.Copy,
                    accum_out=rs[:, k : k + 1],
                )
        eps_col = const.tile([P, 1], f32, name="eps_col")
        nc.vector.memset(eps_col[:], EPS)
        sq = const.tile([P, N2], f32, name="sq")
        nc.scalar.activation(
            sq[:], rs[:], mybir.ActivationFunctionType.Sqrt, bias=eps_col[:]
        )
        dv = const.tile([P, N2], f32, name="dv")
        nc.vector.reciprocal(dv[:], sq[:])

        # lhsT per n2: [ones | dv] -> colsums of Gc (row 0) and Gs (row 1)
        onesdv_f = const.tile([P, N2, 2], f32, name="onesdv_f")
        nc.vector.memset(onesdv_f[:, :, 0:1], 1.0)
        nc.vector.tensor_copy(onesdv_f[:, :, 1:2], dv[:, :, None])
        onesdv = const.tile([P, N2, 2], f32r, name="onesdv")
        nc.vector.tensor_copy(onesdv[:], onesdv_f[:])
        stats_ps = ps_st.tile([2, E], f32, name="stats_ps")
        for k in range(N2):
            nc.tensor.matmul(
                stats_ps[:], onesdv[:, k, :], gc_all[:, k, :],
                start=(k == 0), stop=(k == N2 - 1),
            )
        stats_sb = const.tile([2, E], bf16, name="stats_sb")
        nc.vector.tensor_copy(stats_sb[:], stats_ps[:])

        # transpose stats to column layout [128, ET, 2] = [cs | u0]
        statsT = const.tile([P, ET, 2], f32, name="statsT")
        for j in range(ET):
            tp = ps_t.tile([P, P], bf16, name="sp")[:, 0:2]
            nc.tensor.matmul(
                tp[:], stats_sb[:, j * P : (j + 1) * P], ident_b[0:2, 0:2],
                is_transpose=True,
            )
            nc.vector.tensor_copy(statsT[:, j, :], tp[:])
        de_col = const.tile([P, ET], f32, name="de_col")
        nc.vector.tensor_scalar(
            out=de_col[:], in0=statsT[:, :, 0], scalar1=EPS, scalar2=None,
            op0=mybir.AluOpType.add,
        )
        nc.vector.reciprocal(de_col[:], de_col[:])

        # ub[e-part, j, bf2] = u0[e] * bias-pattern  (added to every zw m-tile)
        ub = const.tile([P, ET, P], f32, name="ub")
        for j in range(ET):
            nc.vector.tensor_scalar(
                out=ub[:, j, :], in0=bias_bc[:], scalar1=statsT[:, j, 1:2],
                scalar2=None, op0=mybir.AluOpType.mult,
            )

        # ---- Gs (bf16) and Gsd -------------------------------------------
        gs_all = big.tile([P, N2, E], bf16, name="gs_all")
        for k in range(N2):
            if k % 2 == 0:
                nc.vector.tensor_scalar(
                    out=gs_all[:, k, :], in0=gc_all[:, k, :], scalar1=dv[:, k : k + 1],
                    scalar2=None, op0=mybir.AluOpType.mult,
                )
            else:
                nc.scalar.activation(
                    gs_all[:, k, :], gc_all[:, k, :],
                    mybir.ActivationFunctionType.Copy, scale=dv[:, k : k + 1],
                )

        # Gsd[e, n-col] = de[e] * Gs[n, e] via PE transpose + scaled evict
        # n-col order is (n2, q): col n2*128+q holds n = q*8 + n2
        gsd_all = big.tile([P, ET, N], bf16, name="gsd_all")

        def gsd_strip(k):
            for j in range(ET):
                tp = ps_t.tile([P, P], bf16, name="sp")
                nc.tensor.matmul(
                    tp[:], gs_all[:, k, j * P : (j + 1) * P], ident_b[:],
                    is_transpose=True,
                )
                if (k * ET + j) % 2 == 0:
                    nc.vector.tensor_scalar(
                        out=gsd_all[:, j, k * P : (k + 1) * P], in0=tp[:],
                        scalar1=de_col[:, j : j + 1], scalar2=None,
                        op0=mybir.AluOpType.mult,
                    )
                else:
                    nc.scalar.activation(
                        gsd_all[:, j, k * P : (k + 1) * P], tp[:],
                        mybir.ActivationFunctionType.Copy,
                        scale=de_col[:, j : j + 1],
                    )

        # ---- main pipeline ----------------------------------------------
        # v_all[e-part, j, bf] = zw + u0*bias  (bf16)
        v_all = big.tile([P, ET, BF], bf16, name="v_all")
        os_r = os_.rearrange("b (p n2) f -> p b n2 f", p=P)

        def mm1(m):
            zps = ps_z.tile([P, E], f32, name="zps")
            for k in range(N2):
                nc.tensor.matmul(
                    zps[:], xs_all[:, k, 2 * m : 2 * m + 2, :], gs_all[:, k, :],
                    start=(k == 0), stop=(k == N2 - 1),
                )
            return zps

        def wmm(m, zps):
            zt = ztp.tile([P, E], bf16, name="zt")
            nc.scalar.copy(zt[:], zps[:])
            wps = ps_w.tile([P, E], f32, name="wps")
            for j in range(ET):
                nc.tensor.matmul(
                    wps[:, j * P : (j + 1) * P], zt[:, j * P : (j + 1) * P], bdw[:],
                    start=True, stop=True,
                )
            # v = ub + zw for all 4 j-blocks in one DVE op
            nc.vector.scalar_tensor_tensor(
                out=v_all[:, :, m * P : (m + 1) * P],
                in0=ub[:],
                scalar=1.0,
                in1=wps[:].rearrange("p (j c) -> p j c", j=ET),
                op0=mybir.AluOpType.mult,
                op1=mybir.AluOpType.add,
            )

        def mm2(c):
            c0, c1 = CHUNKS[c]
            nb = (c1 - c0) // F  # batch entries in this chunk
            ob = osb.tile([P, 8, N2, F], f32, name="ob")
            for k in range(N2):
                ops = ps_o.tile([P, 512], f32, name="ops")[:, 0 : c1 - c0]
                for j in range(ET):
                    nc.tensor.matmul(
                        ops[:], gsd_all[:, j, k * P : (k + 1) * P],
                        v_all[:, j, c0:c1],
                        start=(j == 0), stop=(j == ET - 1),
                    )
                dst = ob[:, 0:nb, k, :]
                src = ops[:].rearrange("p (c f) -> p c f", f=F)
                if k % 2 == 0:
                    nc.scalar.copy(dst, src)
                else:
                    nc.vector.tensor_copy(dst, src)
            nc.scalar.dma_start(
                os_r[:, c0 // F : c1 // F], ob[:, 0:nb, :, :]
            )

        for k in range(N2):
            gsd_strip(k)
        zps_prev = mm1(0)
        for m in range(1, MT):
            zps = mm1(m)
            wmm(m - 1, zps_prev)
            zps_prev = zps
            if m - 1 in CHUNK_AFTER:
                mm2(CHUNK_AFTER[m - 1])
        wmm(MT - 1, zps_prev)
        mm2(CHUNK_AFTER[MT - 1])

    nc.finalize()
    return nc


_NC = None


def _get_nc():
    global _NC
    if _NC is None:
        _NC = _build_nc()
    return _NC


def kernel(x, G, G1, weight, bias):
    nc = _get_nc()
    x = np.ascontiguousarray(x, dtype=np.float32)
    G = np.ascontiguousarray(G, dtype=np.float32)
    G1 = np.ascontiguousarray(G1, dtype=np.float32)
    weight = np.ascontiguousarray(weight, dtype=np.float32)
    bias = np.ascontiguousarray(bias, dtype=np.float32)

    in_maps = []
    for c in range(T):
        in_maps.append(
            {
                "xs": x[c * B : (c + 1) * B],
                "g": G,
                "g1": np.ascontiguousarray(G1[c]),
                "w": weight,
                "b": bias,
            }
        )
    res = bass_utils.run_bass_kernel_spmd(nc, in_maps, core_ids=list(range(T)))
    return np.concatenate([r["os"] for r in res.results], axis=0)
